# revision 29
# baseline (speedup 1.0000x reference)
"""Distributed MoE layer (16384 tokens, hidden 1024, ffn 4096, 8 experts, top-2)
on 8 TRN2 NeuronCores, expert-parallel with pairwise load balancing.

Host: router (replicated math, same semantics as the jax reference) + token
dispatch by target expert (the "all-to-all") + weighted combine.

Experts are paired big-with-small; each pair maps to two cores, each core
processing half of each paired expert's tokens: region A = [0, RA) runs the
"hi" expert, region B = [RA, RA+RB) the "lo" expert. This balances every
core to C = RA + RB ~ (max_hi + max_lo)/2 tokens instead of padding all
cores to the largest expert count.

Device: y = coeff * (gelu(x @ w1) @ w2) in bf16 operands, fp32 PSUM.
"""

import sys

if "/opt/trn_rl_repo" not in sys.path:
    sys.path.insert(0, "/opt/trn_rl_repo")

import numpy as np
import ml_dtypes

import concourse.mybir as mybir
import concourse.tile as tile
from concourse import bacc
from concourse.bass_utils import run_bass_kernel_spmd

N_TOKENS = 16384
HIDDEN = 1024
FFN = 4096
N_EXPERTS = 8
TOP_K = 2
P = 128
TG = 512  # phase-1 token group / phase-2 span width
PG = 256  # phase-2 psum subgroup (4 banks, double-buffered by the pool)

FP32 = mybir.dt.float32
BF16 = mybir.dt.bfloat16
NP_BF16 = ml_dtypes.bfloat16

_KERNEL_CACHE: dict[tuple, object] = {}


def _split_groups(base, size, step):
    out = []
    b = base
    while b < base + size:
        w = min(step, base + size - b)
        out.append((b, w))
        b += w
    return out


def _build(RA: int, RB: int, cover_a: int, cover_b: int):
    """Per-core two-region FFN kernel. Region A=[0,RA) uses w1a/w2a,
    region B=[RA,RA+RB) uses w1b/w2b. cover_a/cover_b are the maximum
    real (non-padding) token counts over all cores: phase 1 only computes
    and spills h for covered columns, and phase 2 only loads those; psum
    rows over uncovered columns see stale finite SBUF data and are zeroed
    by their coeff of 0."""
    assert RA % P == 0 and RB % P == 0
    assert 0 < cover_a <= RA and 0 < cover_b <= RB
    C = RA + RB
    KH = HIDDEN // P  # 8 hidden chunks
    KF = FFN // P  # 32 ffn chunks
    NEARLY = 24  # w2a chunks preloaded during phase 1
    HPRE = KF  # span-0 hf chunks prefetched during phase 1

    nc = bacc.Bacc("TRN2", target_bir_lowering=False, debug=False)
    gelu = mybir.ActivationFunctionType.Gelu_apprx_tanh

    with tile.TileContext(nc) as tc:
        with tc.tile_pool(name="dram", bufs=1, space="DRAM") as dram:
            xt = dram.tile([HIDDEN, C], BF16, kind="ExternalInput", uniquify=False, name="xt")
            w1a = dram.tile([KF, P, KH, P], BF16, kind="ExternalInput", uniquify=False, name="w1a")
            w1b = dram.tile([KF, P, KH, P], BF16, kind="ExternalInput", uniquify=False, name="w1b")
            w2a = dram.tile([FFN, HIDDEN], BF16, kind="ExternalInput", uniquify=False, name="w2a")
            w2b = dram.tile([FFN, HIDDEN], BF16, kind="ExternalInput", uniquify=False, name="w2b")
            cf = dram.tile([C, 1], FP32, kind="ExternalInput", uniquify=False, name="cf")
            y = dram.tile([C, HIDDEN], FP32, kind="ExternalOutput", uniquify=False, name="y")
            ht = dram.tile([FFN, C], BF16, kind="Internal", uniquify=False, name="ht")

            xt3 = xt[:].rearrange("(ko p) n -> p ko n", p=P)  # [128, 8, C]
            w2a3 = w2a[:].rearrange("(fo p) h -> p fo h", p=P)  # [128, 32, 1024]
            w2b3 = w2b[:].rearrange("(fo p) h -> p fo h", p=P)
            ht3 = ht[:].rearrange("(fo p) n -> p fo n", p=P)  # [128, 32, C]

            # phase-1 pair list: groups of <=TG tokens per region, paired so
            # consecutive matmuls share each w1 stationary chunk. Use an even
            # number of near-equal groups so no pair is a lone narrow group
            # (narrow f-slots overload the scalar queue and starve the PE).
            def make_pairs(base, size):
                n = -(-size // TG)
                if n > 1 and n % 2 == 1:
                    n += 1
                w = -(-(-(-size // n)) // 8) * 8
                gs = _split_groups(base, size, w)
                return [gs[i : i + 2] for i in range(0, len(gs), 2)]

            pairs_a = make_pairs(0, cover_a)
            pairs_b = make_pairs(RA, cover_b)
            all_pairs = [(0, p) for p in pairs_a] + [(1, p) for p in pairs_b]
            last_a_idx = len(pairs_a) - 1
            last_idx = len(all_pairs) - 1

            # phase-2 spans: TG-wide hf windows, each holding <=2 psum
            # subgroups of PG tokens. Partial spans go last so the final
            # drain (vector muls + y writes after the last matmul) is short.
            spans_a = _split_groups(0, RA, TG)
            spans_b = _split_groups(RA, RB, TG)
            spans = (
                [(0, s) for s in spans_a if s[1] == TG]
                + [(1, s) for s in spans_b if s[1] == TG]
                + [(1, s) for s in spans_b if s[1] < TG]
                + [(0, s) for s in spans_a if s[1] < TG]
            )

            w2ts = [None] * KF  # region-A w2 chunks (preloaded + streamed)
            hf0 = [None] * KF  # span-0 hf chunks prefetched in phase 1
            w2e_ctx = tc.tile_pool(name="w2e", bufs=1)
            w2e = w2e_ctx.__enter__()
            hip_ctx = tc.tile_pool(name="hip", bufs=40)
            hip = hip_ctx.__enter__()

            def load_hf(f, sbase, sw):
                t = hip.tile([P, TG], BF16, name="hf", tag="hf")
                cov_end = cover_a if sbase < RA else RA + cover_b
                w = min(sw, cov_end - sbase)
                if w > 0:
                    nc.gpsimd.dma_start(t[:, :w], ht3[:, f, sbase : sbase + w])
                return t

            # ---- phase 1: hT = gelu(w1.T @ x.T), spilled to DRAM (bf16) ----
            with tc.tile_pool(name="w1p", bufs=1) as w1p, tc.tile_pool(
                name="xp", bufs=1
            ) as xp, tc.tile_pool(name="hp", bufs=5) as hp, tc.tile_pool(
                name="pp1", bufs=4, space="PSUM"
            ) as pp1:
                xslot = [0]

                def load_xg(base, w):
                    s = xslot[0]
                    xslot[0] = (s + 1) % 4
                    t = xp.tile([P, KH, TG], BF16, name=f"xg{s}", tag=f"xg{s}")
                    qs = [nc.sync, nc.gpsimd, nc.sync, nc.gpsimd]
                    for j in range(0, KH, 2):
                        qs[j // 2].dma_start(
                            t[:, j : j + 2, :w],
                            xt3[:, j : j + 2, base : base + w],
                        )
                    return t

                def load_pair0(pair):
                    # two-chunk batched loads in matmul consumption order
                    # (k-major, group-minor) round-robined over three queues
                    # so the first matmuls start as early as possible
                    ts = []
                    for gi in range(len(pair)):
                        s = xslot[0]
                        xslot[0] = (s + 1) % 4
                        ts.append(
                            xp.tile([P, KH, TG], BF16, name=f"xg{s}", tag=f"xg{s}")
                        )
                    qs = [nc.sync, nc.scalar, nc.gpsimd]
                    qi = 0
                    for j in range(0, KH, 2):
                        for gi, (base, w) in enumerate(pair):
                            qs[qi % 3].dma_start(
                                ts[gi][:, j : j + 2, :w],
                                xt3[:, j : j + 2, base : base + w],
                            )
                            qi += 1
                    return ts

                w1ts = [None] * KF

                def load_w1(src, f, q=None):
                    w1t = w1p.tile([P, KH, P], BF16, name=f"w1t{f}", tag=f"w1t{f}")
                    (q or nc.gpsimd).dma_start(w1t[:], src[f])
                    w1ts[f] = w1t

                def gelu_spill(ps_h, base, w, f):
                    hg = hp.tile([P, TG], BF16, name="hg", tag="hg")
                    nc.scalar.activation(hg[:, :w], ps_h[:, :w], gelu)
                    nc.sync.dma_start(ht3[:, f, base : base + w], hg[:, :w])

                # queue the first w1 chunk, x for the first pair, then the
                # remaining region-A w1 chunks
                # first w1 chunk ahead of everything; a few early chunks go
                # on the still-idle scalar queue so the stream starts ~2x
                # ahead of the PE before gelu activations claim that queue
                load_w1(w1a, 0)
                xnext = load_pair0(all_pairs[0][1])
                for f in range(1, KF):
                    load_w1(w1a, f, q=nc.scalar if f in (1, 3, 5) else None)

                for pi, (ri, pair) in enumerate(all_pairs):
                    xgs = xnext
                    if pi + 1 < len(all_pairs):
                        # prefetch the next pair's x a full pair ahead
                        xnext = [load_xg(b, w) for (b, w) in all_pairs[pi + 1][1]]
                    for f in range(KF):
                        pss = [
                            pp1.tile([P, TG], FP32, name="ps_h", tag=f"ps_h{gi}")
                            for gi in range(len(pair))
                        ]
                        for k in range(KH):
                            for gi, (base, w) in enumerate(pair):
                                nc.tensor.matmul(
                                    pss[gi][:, :w],
                                    lhsT=w1ts[f][:, k, :],
                                    rhs=xgs[gi][:, k, :w],
                                    start=(k == 0),
                                    stop=(k == KH - 1),
                                )
                        for gi, (base, w) in enumerate(pair):
                            gelu_spill(pss[gi], base, w, f)
                        if pi == last_a_idx:
                            # stream region-B w1 into the freed chunk slots
                            load_w1(w1b, f)
                        if pi == last_idx:
                            # preload early region-A w2 chunks
                            fe = f - (KF - NEARLY)
                            if fe >= 0:
                                w2t = w2e.tile(
                                    [P, HIDDEN], BF16, name=f"w2t{fe}", tag=f"w2t{fe}"
                                )
                                nc.gpsimd.dma_start(w2t[:], w2a3[:, fe, :])
                                w2ts[fe] = w2t
                # prefetch span 0's hf so phase 2 starts without a stall.
                # These depend on span-0 spills, done long ago.
                sb0, sw0 = spans[0][1]
                for f in range(HPRE):
                    hf0[f] = load_hf(f, sb0, sw0)

            # ---- phase 2: y = coeff * (hT.T @ w2) ----
            with tc.tile_pool(name="w2p", bufs=1) as w2p, tc.tile_pool(
                name="w2q", bufs=1
            ) as w2q, tc.tile_pool(name="cp", bufs=4) as cp, tc.tile_pool(
                name="yp", bufs=4
            ) as yp, tc.tile_pool(name="pp2", bufs=2, space="PSUM") as pp2:
                w2bts = [None] * KF

                def load_w2a(f):
                    w2t = w2p.tile([P, HIDDEN], BF16, name=f"w2t{f}", tag=f"w2t{f}")
                    nc.gpsimd.dma_start(w2t[:], w2a3[:, f, :])
                    w2ts[f] = w2t

                def load_w2b(f):
                    w2t = w2q.tile([P, HIDDEN], BF16, name=f"w2bt{f}", tag=f"w2bt{f}")
                    nc.gpsimd.dma_start(w2t[:], w2b3[:, f, :])
                    w2bts[f] = w2t

                sidx = 0
                for ri, (sbase, sw) in spans:
                    wts_r = w2ts if ri == 0 else w2bts
                    hfs = hf0 if sidx == 0 else [None] * KF
                    for si, (base, w) in enumerate(_split_groups(sbase, sw, PG)):
                        nt = (w + P - 1) // P
                        cts = []
                        for tt in range(nt):
                            ct = cp.tile([P, 1], FP32, name="ct", tag=f"ct{tt}")
                            nc.gpsimd.dma_start(
                                ct[:], cf[base + tt * P : base + (tt + 1) * P, :]
                            )
                            cts.append(ct)
                        psy = [
                            [
                                pp2.tile([P, TG], FP32, name="psy", tag=f"psy{tt}_{nh}")
                                for nh in range(HIDDEN // TG)
                            ]
                            for tt in range(nt)
                        ]
                        off = base - sbase
                        for f in range(KF):
                            if sidx == 0 and si == 0 and NEARLY + f < KF:
                                load_w2a(NEARLY + f)
                            if 1 <= sidx <= 4 and si == 0 and f < KF // 4:
                                load_w2b((sidx - 1) * (KF // 4) + f)
                            if si == 0 and hfs[f] is None:
                                hfs[f] = load_hf(f, sbase, sw)
                            for tt in range(nt):
                                for nh in range(HIDDEN // TG):
                                    nc.tensor.matmul(
                                        psy[tt][nh][:],
                                        lhsT=hfs[f][:, off + tt * P : off + (tt + 1) * P],
                                        rhs=wts_r[f][:, nh * TG : (nh + 1) * TG],
                                        start=(f == 0),
                                        stop=(f == KF - 1),
                                    )
                        # drain psum on both vector and scalar (scalar is
                        # idle in phase 2), halving the per-group drain tail
                        for tt in range(nt):
                            for nh in range(HIDDEN // TG):
                                yt = yp.tile([P, TG], FP32, name="yt", tag=f"yt{tt}_{nh}")
                                if (2 * tt + nh) % 2 == 0:
                                    nc.vector.tensor_scalar_mul(
                                        yt[:], psy[tt][nh][:], cts[tt][:]
                                    )
                                else:
                                    nc.scalar.activation(
                                        yt[:],
                                        psy[tt][nh][:],
                                        mybir.ActivationFunctionType.Copy,
                                        scale=cts[tt][:],
                                    )
                                yq = nc.sync if (2 * tt + nh) % 2 == 0 else nc.scalar
                                yq.dma_start(
                                    y[
                                        base + tt * P : base + (tt + 1) * P,
                                        nh * TG : (nh + 1) * TG,
                                    ],
                                    yt[:],
                                )
                    sidx += 1

            hip_ctx.__exit__(None, None, None)
            w2e_ctx.__exit__(None, None, None)

    nc.compile()
    return nc


def _get_kernel(RA: int, RB: int, cover_a: int, cover_b: int):
    key = (RA, RB, cover_a, cover_b)
    if key not in _KERNEL_CACHE:
        _KERNEL_CACHE[key] = _build(RA, RB, cover_a, cover_b)
    return _KERNEL_CACHE[key]


def _route(x: np.ndarray, w_router: np.ndarray):
    """Replicates the reference router: softmax -> top-2 -> renormalize."""
    logits = x @ w_router  # [N, E] fp32
    order = np.argsort(-logits, axis=1, kind="stable")
    i1, i2 = order[:, 0], order[:, 1]
    l64 = logits.astype(np.float64)
    l64 -= l64.max(axis=1, keepdims=True)
    e = np.exp(l64)
    p = e / e.sum(axis=1, keepdims=True)
    rows = np.arange(x.shape[0])
    p1 = p[rows, i1]
    p2 = p[rows, i2]
    s = p1 + p2
    return i1, i2, (p1 / s).astype(np.float32), (p2 / s).astype(np.float32)


def _pack_w1(w1e):
    return np.ascontiguousarray(
        w1e.reshape(HIDDEN // P, P, FFN // P, P).transpose(2, 1, 0, 3)
    ).astype(NP_BF16)


def _align(v):
    return -(-v // P) * P


def _prepare(x, w_router, w1, w2):
    """Route on host, build per-core bf16 input maps with pairwise expert
    balancing. Returns (in_maps, placements, RA, RB)."""
    n = x.shape[0]
    i1, i2, c1, c2 = _route(x, w_router)

    slot_expert = np.concatenate([i1, i2])
    slot_coeff = np.concatenate([c1, c2])
    slot_token = np.concatenate([np.arange(n), np.arange(n)])
    counts = np.bincount(slot_expert, minlength=N_EXPERTS)

    order = np.argsort(slot_expert, kind="stable")
    tok_sorted = slot_token[order]
    coef_sorted = slot_coeff[order]
    starts = np.concatenate([[0], np.cumsum(counts)])
    tok_e = [tok_sorted[starts[e] : starts[e + 1]] for e in range(N_EXPERTS)]
    coe_e = [coef_sorted[starts[e] : starts[e + 1]] for e in range(N_EXPERTS)]

    # pair the biggest expert with the smallest, 2nd with 2nd-smallest, ...
    desc = np.argsort(-counts, kind="stable")
    pairs = [(int(desc[k]), int(desc[N_EXPERTS - 1 - k])) for k in range(N_EXPERTS // 2)]
    RA = _align(max((counts[hi] + 1) // 2 for hi, _ in pairs))
    RB = _align(max((counts[lo] + 1) // 2 for _, lo in pairs))

    xb = x.astype(NP_BF16)
    w1p = {}
    w2p = {}
    for e in range(N_EXPERTS):
        w1p[e] = _pack_w1(w1[e])
        w2p[e] = np.ascontiguousarray(w2[e]).astype(NP_BF16)

    in_maps = []
    placements = []  # per core: list of (expert, token_idx_array, region_base)
    C = RA + RB
    cover_a = max((counts[hi] + 1) // 2 for hi, _ in pairs)
    cover_b = max((counts[lo] + 1) // 2 for _, lo in pairs)
    for hi, lo in pairs:
        ha = (counts[hi] + 1) // 2
        la = (counts[lo] + 1) // 2
        splits = [
            ((tok_e[hi][:ha], coe_e[hi][:ha]), (tok_e[lo][:la], coe_e[lo][:la])),
            ((tok_e[hi][ha:], coe_e[hi][ha:]), (tok_e[lo][la:], coe_e[lo][la:])),
        ]
        for (ta, ca), (tb, cb) in splits:
            xtile = np.zeros((HIDDEN, C), dtype=NP_BF16)
            cfv = np.zeros((C, 1), dtype=np.float32)
            xtile[:, : len(ta)] = xb[ta].T
            cfv[: len(ta), 0] = ca
            xtile[:, RA : RA + len(tb)] = xb[tb].T
            cfv[RA : RA + len(tb), 0] = cb
            in_maps.append(
                {
                    "xt": xtile,
                    "w1a": w1p[hi],
                    "w1b": w1p[lo],
                    "w2a": w2p[hi],
                    "w2b": w2p[lo],
                    "cf": cfv,
                }
            )
            placements.append([(hi, ta, 0), (lo, tb, RA)])
    return in_maps, placements, RA, RB, cover_a, cover_b


def kernel(x, w_router, w1, w2):
    x = np.ascontiguousarray(x, dtype=np.float32)
    w_router = np.ascontiguousarray(w_router, dtype=np.float32)
    w1 = np.ascontiguousarray(w1, dtype=np.float32)
    w2 = np.ascontiguousarray(w2, dtype=np.float32)
    n = x.shape[0]

    in_maps, placements, RA, RB, cover_a, cover_b = _prepare(x, w_router, w1, w2)
    nc = _get_kernel(RA, RB, cover_a, cover_b)
    res = run_bass_kernel_spmd(nc, in_maps, core_ids=list(range(N_EXPERTS)))

    out = np.zeros((n, HIDDEN), dtype=np.float32)
    for c in range(N_EXPERTS):
        yc = res.results[c]["y"]
        for _, tk, rb in placements[c]:
            out[tk] += yc[rb : rb + len(tk)]
    return out
